# revision 49
# baseline (speedup 1.0000x reference)
"""Causal single-head attention block on 8 TRN2 NeuronCores.

Problem: B=8, T=1024, D=1024 fp32.
    q = x @ w_q.T + b_q ; k, v likewise
    scores = (q @ k.T) / sqrt(D), causal mask, softmax
    out = (softmax @ v) @ w_o.T + b_o

Sharding: pure data-parallel - core c computes batch element c. Weights are
replicated. No collectives.

Algebraic restructuring (removes 2 of the 5 D x D GEMMs):
  scores_ij = q_i . k_j = x_i^T (Wq^T Wk) x_j + (Wk^T bq) . x_j + f(i)
  where f(i) is constant per query row and cancels in softmax. So with
  A = Wq^T Wk and u = Wk^T bq (precomputed host-side from the weights):
      Y = X A + 1 u^T        (one GEMM; replaces the q AND k projections)
      S = Y X^T / sqrt(D)    (the score GEMM, X itself is the stationary side)
  Similarly, since softmax rows sum to 1, the v bias feeds straight through:
      out = attn @ (X Wv^T + 1 bv^T) @ Wo^T + 1 bo^T
          = (attn @ X) @ C^T + 1 b'^T,  C = Wo Wv, b' = Wo bv + bo.
  Total per-core MACs drop from 5.36e9 to 3.28e9.

fp8 hi/lo DoubleRow matmuls (~1.33x on the remaining big GEMMs):
  Every big-GEMM operand is split as w = hi + lo with hi = fp8(w),
  lo = fp8(w - hi); dropping only the lo*lo term, each 128-deep contraction
  chunk needs 3 fp8 sub-matmuls, packed 2-per-instruction with
  MatmulPerfMode.DoubleRow (0.5 PE cycles/row):
      DR_A(d,d+1) = hi_d*hi_d + hi_{d+1}*hi_{d+1}   (cross-chunk pair)
      DR_B(d)     = hi_d*lo_d + lo_d*hi_d           (within-chunk pair)
  so 8 chunks take 12 DR instructions = 6N cycles vs 8N for bf16, at
  bf16-level accuracy. Scaling (all folded away exactly by the ONES_VAL
  rowsum constant and the exp scale): A is pre-scaled by 16 (|16 Y'| < 123
  across all 8 batch inputs, vs the e4m3 max of 240) and C by 32 (fp8
  subnormal headroom); the exp() epilogue absorbs 1/16/sqrt(D) into its
  scale and an extra -ln(32) bias plus a 1/4 scale on the Z' = attn @ X
  hi/lo split keep exp outputs and Z' inside e4m3 range (|Z'/4| < 97).

  attn weights are split hi/lo only for the full-width (non-diagonal)
  chunk-1 tiles, where DoubleRow pairs exist; the softmax denominators for
  those tiles sum the same hi+lo pair via a DoubleRow ones-matmul, so the
  normalization stays exactly consistent with the numerator. Plain-fp8
  (non-split) attn weights would NOT work: softmax normalization does not
  attenuate their quantization error and the error budget blows.

  Rows 0-127 have tiny softmax denominators; after the final 1/rowsum
  scaling the fp8 subnormal floor on Z' would blow up relatively, so that
  row block runs through a parallel bf16 path (otb/ctb) instead.

Device-side layout strategy (transpose-free), as in the baseline:
  - scores are computed TRANSPOSED: scoresT[tk, tq], k-side stationary.
  - softmax denominators via tiny N=1 matmuls against a constant column.
  - causal structure: for tq-chunk c (512 wide), only tk-tiles i <= 4c+3 are
    computed; partially-valid tiles use shortened matmuls and the diagonal
    128x128 block gets an additive -1e30 upper-tri mask.
"""

import os
import numpy as np
import ml_dtypes

BF = ml_dtypes.bfloat16
F8 = ml_dtypes.float8_e4m3

B, T, D = 8, 1024, 1024
P = 128
ND = D // P          # 8 d-tiles / e-tiles
NT = T // P          # 8 t-tiles
CH = 512             # matmul moving free-dim (one PSUM bank of fp32)
NCH = T // CH        # 2 tq-chunks
SM_SCALE = float(D) ** -0.5
A_SCALE = 16.0       # host pre-scale on A = Wq^T Wk (fp8 range: |16 Y'| < 123)
C_SCALE = 32.0       # host pre-scale on C = Wo Wv (fp8 subnormal headroom)
Z_SCALE = 0.25       # extra scale on the Z' fp8 hi/lo split (|Z'/4| < 97)
ONES_VAL = 8.0       # rowsum column constant; folds all scales away exactly
LN32 = 3.4657359027997265
EXP_SCALE = SM_SCALE / A_SCALE   # 1/512, applied inside the exp activation
MASK_VAL = -1.0e30

_CACHE = {}


def _build_program():
    import concourse.bass as bass
    import concourse.mybir as mybir
    import concourse.tile as tile
    from concourse.bass import ts

    F32 = mybir.dt.float32
    BF16 = mybir.dt.bfloat16
    FP8 = mybir.dt.float8e4
    AF = mybir.ActivationFunctionType
    ALU = mybir.AluOpType
    DR = mybir.MatmulPerfMode.DoubleRow

    nc = bass.Bass()

    # x arrives pre-split/pre-tiled from the host:
    #   xT8[c, p, dd, s, t] = split_s(x.T)[128 dd + p, 512 c + t], s: 0=hi 1=lo
    #   xnat[p, j, e] = x[128 j + p, e] (bf16, stationary side of attn @ X)
    xT8_d = nc.declare_dram_parameter("xT8", [NCH, P, ND, 2, CH], FP8, isOutput=False)
    xnat_d = nc.declare_dram_parameter("xnat", [P, NT, D], BF16, isOutput=False)
    # fp8 hi/lo copy of x rows 0-767 (stationary side of the DoubleRow
    # attn @ X tiles); s: 0=hi 1=lo
    xnat8_d = nc.declare_dram_parameter("xnat8", [P, 6, 2, D], FP8, isOutput=False)
    # A32t[ee, p, dd, s, el] = split_s(32 Wq^T Wk)[128 dd + p, 128 ee + el],
    # s: 0=lo 1=hi  (one contiguous 2 KiB run per partition per ee)
    a_d = nc.declare_dram_parameter("a32t", [ND, P, ND, 2, P], FP8, isOutput=False)
    # ct32[dd, p, s, e] = split_s(32 (Wo Wv)^T)[128 dd + p, e], s: 0=lo 1=hi
    ct_d = nc.declare_dram_parameter("ct32", [ND, P, 2, D], FP8, isOutput=False)
    # bf16 copy of 32 C^T for the early-row (c=0, jj=0) output groups
    ctb_d = nc.declare_dram_parameter("ctb", [P, ND, D], BF16, isOutput=False)
    u32_d = nc.declare_dram_parameter("u32T", [P, ND], F32, isOutput=False)
    bob_d = nc.declare_dram_parameter("bob", [P, D], BF16, isOutput=False)
    mask_d = nc.declare_dram_parameter("maskT", [P, P], F32, isOutput=False)
    out_d = nc.declare_dram_parameter("out", [T, D], F32, isOutput=True)

    # Default slot order touches chunks 6/7 last: their operands are the
    # ones still in flight (last DMA piece, or the epilogue of the previous
    # phase's final tile), so the group starts without waiting on them.
    DR_ORDER = [("hh", 0), ("hh", 2), ("hh", 4), ("x", 0), ("x", 1),
                ("x", 2), ("x", 3), ("x", 4), ("x", 5), ("hh", 6),
                ("x", 6), ("x", 7)]
    # For the very first group the xT chunk-0 pieces land dd-pair by dd-pair
    # across both DMA lanes: consume in arrival order.
    DR_ORDER_FIRST = [("hh", 0), ("x", 0), ("x", 1), ("hh", 6), ("x", 6),
                      ("x", 7), ("hh", 2), ("x", 2), ("x", 3), ("hh", 4),
                      ("x", 4), ("x", 5)]

    def dr_group(ps, stat, mov, order=None):
        """Emit the 12-instruction DoubleRow group for one 1024-deep
        contraction; stat/mov are callables returning the (2, free)-shaped
        slot APs:
            stat('hh', d) / mov('hh', d): hi slots of chunks d and d+1
            stat('x', d)  / mov('x', d) : the two cross slots of chunk d
        """
        order = DR_ORDER if order is None else order
        for k, (kind, d) in enumerate(order):
            nc.tensor.matmul(ps, stat(kind, d), mov(kind, d),
                             start=(k == 0), stop=(k == len(order) - 1),
                             perf_mode=DR)

    with tile.TileContext(nc) as tc:
        with (
            tc.tile_pool(name="pers", bufs=1) as pers,
            tc.tile_pool(name="psum", bufs=2, space="PSUM") as psp,
        ):
            # ---- persistent SBUF tensors --------------------------------
            xT_sb = pers.tile([P, ND, 2, T], FP8)     # s: 0=hi 1=lo
            xnat_sb = pers.tile([P, NT, D], BF16)
            xnat8_sb = pers.tile([P, 6, 2, D], FP8)   # s: 0=hi 1=lo
            y_sb = pers.tile([P, ND, 2, T], FP8)      # s: 0=lo 1=hi
            a_sb = pers.tile([P, ND, ND, 2, P], FP8)  # [p, ee, dd, s, el] 0=lo 1=hi
            ct_sb = pers.tile([P, ND, 2, D], FP8)     # s: 0=lo 1=hi
            ct_b16 = pers.tile([P, ND, D], BF16)
            u32 = pers.tile([P, ND], F32)
            bob = pers.tile([P, D], BF16)
            maskT = pers.tile([P, P], F32)
            ones_c = pers.tile([P, 2], BF16)
            ones8 = pers.tile([P, 2, 1], FP8)
            negln32 = pers.tile([P, 1], F32)
            r_all = pers.tile([P, NT], F32)

            with tc.tile_pool(name="attn_tmp", bufs=3) as atm:
                # ---- DMAs: two serial lanes (SP sequencer + Pool/SWDGE).
                # Critical path: A[ee=0] + xT hi chunk 0 feed the first
                # DoubleRow group.
                # xT chunk 0 lands as dd-pair pieces (hi+lo together) split
                # across both DMA lanes, in the order DR_ORDER_FIRST consumes
                # them; DMA latency is ~1.9 us flat so small leading pieces
                # start the first DoubleRow group ~1 us earlier.
                nc.sync.dma_start(a_sb[:, 0], a_d[0])
                nc.gpsimd.dma_start(
                    xT_sb[:, 0:2, :, ts(0, CH)], xT8_d[0][:, 0:2]
                )
                nc.sync.dma_start(
                    xT_sb[:, 6:ND, :, ts(0, CH)], xT8_d[0][:, 6:ND]
                )
                nc.gpsimd.dma_start(
                    xT_sb[:, 2:4, :, ts(0, CH)], xT8_d[0][:, 2:4]
                )
                nc.gpsimd.dma_start(
                    xT_sb[:, 4:6, :, ts(0, CH)], xT8_d[0][:, 4:6]
                )
                nc.sync.dma_start(u32, u32_d[:, :])
                for ee in range(1, 4):
                    nc.sync.dma_start(a_sb[:, ee], a_d[ee])
                nc.gpsimd.dma_start(xT_sb[:, :, 0, ts(1, CH)], xT8_d[1][:, :, 0])
                nc.gpsimd.dma_start(xT_sb[:, :, 1, ts(1, CH)], xT8_d[1][:, :, 1])
                for ee in range(4, ND):
                    nc.gpsimd.dma_start(a_sb[:, ee], a_d[ee])
                nc.sync.dma_start(maskT, mask_d[:, :])
                for dd in range(ND):
                    nc.sync.dma_start(ct_sb[:, dd], ct_d[dd])
                nc.sync.dma_start(bob, bob_d[:, :])
                nc.gpsimd.dma_start(xnat_sb, xnat_d[:, :])
                nc.gpsimd.dma_start(xnat8_sb, xnat8_d[:, :])
                nc.gpsimd.dma_start(ct_b16, ctb_d[:, :])


                # ---- PE warm-up: dummy matmuls overlap the initial DMA
                # fill and spin the p-state clock up; result never read. The
                # dummy activation pre-loads the ScalarE exp table (the set
                # containing exp also contains identity, so no reloads).
                warm_in = atm.tile([P, P], BF16, tag="warm", bufs=1)
                nc.vector.memset(warm_in, 0.0)
                nc.vector.memset(ones_c, float(ONES_VAL))
                nc.vector.memset(ones8, float(ONES_VAL))
                nc.vector.memset(negln32, -LN32)
                act_warm = atm.tile([P, 1], F32, tag="warma", bufs=1)
                nc.scalar.activation(
                    act_warm, warm_in[:, :1], AF.Exp, bias=0.0, scale=1.0
                )
                warm_ps = psp.tile([P, CH], F32, tag="mm512", bufs=3)
                for _ in range(15):
                    nc.tensor.matmul(
                        warm_ps[:, :P], warm_in, warm_in, start=True, stop=True
                    )

                # ---- phase A: Y = 32(X A + 1 u^T), split hi/lo ----------
                def a_stat(ee):
                    def f(kind, d):
                        if kind == "hh":
                            return a_sb[:, ee, d : d + 2, 1, :]
                        return a_sb[:, ee, d, 0:2, :]
                    return f

                def xmov(c):
                    lo, hi = CH * c, CH * (c + 1)
                    def f(kind, d):
                        if kind == "hh":
                            return xT_sb[:, d : d + 2, 0, lo:hi]
                        return xT_sb[:, d, 0:2, lo:hi]
                    return f

                def run_yproj(c):
                    for ee in range(ND):
                        ps = psp.tile([P, CH], F32, tag="mm512", bufs=3)
                        dr_group(ps, a_stat(ee), xmov(c),
                                 DR_ORDER_FIRST if c == 0 and ee == 0 else None)
                        # yhi then ylo = (ps + u) - yhi; PSUM -> SBUF fp8
                        nc.scalar.activation(
                            y_sb[:, ee, 1, ts(c, CH)],
                            ps,
                            AF.Identity,
                            bias=u32[:, ee : ee + 1],
                            scale=1.0,
                        )
                        nc.vector.scalar_tensor_tensor(
                            y_sb[:, ee, 0, ts(c, CH)],
                            ps,
                            u32[:, ee : ee + 1],
                            y_sb[:, ee, 1, ts(c, CH)],
                            ALU.add,
                            ALU.subtract,
                        )
                        if c == 0 and ee == 0:
                            # bridge until the xT lo-half / chunk-1 DMAs land
                            for _ in range(6):
                                nc.tensor.matmul(
                                    warm_ps[:, :P],
                                    warm_in,
                                    warm_in,
                                    start=True,
                                    stop=True,
                                )

                # ---- phases B/C/D per tq-chunk --------------------------
                at_tiles = {}

                def run_scores(c):  # phase B
                    n_tk = 4 * (c + 1)
                    at8 = atm.tile([P, 6, 2, CH], FP8, tag="at8", bufs=2)
                    tiles, offs = [], []
                    for i in range(n_tk):
                        off = max(0, P * i - CH * c)
                        offs.append(off)
                        lo, hi = CH * c + off, CH * (c + 1)
                        ps = psp.tile([P, CH], F32, tag="sc", bufs=2)

                        def stat(kind, d, _i=i):
                            if kind == "hh":
                                return xT_sb[:, d : d + 2, 0, ts(_i, P)]
                            return xT_sb[:, d, 0:2, ts(_i, P)]

                        def mov(kind, d, _lo=lo, _hi=hi):
                            if kind == "hh":
                                return y_sb[:, d : d + 2, 1, _lo:_hi]
                            return y_sb[:, d, 0:2, _lo:_hi]

                        dr_group(ps[:, off:], stat, mov)
                        if i >= 4 * c:
                            # diagonal 128x128 block: additive upper-tri mask
                            nc.vector.tensor_add(
                                ps[:, off : off + P], ps[:, off : off + P], maskT
                            )
                        at = atm.tile([P, CH], BF16, tag="at", bufs=9)
                        nc.scalar.activation(
                            at[:, off:], ps[:, off:], AF.Exp,
                            bias=negln32[:, 0:1], scale=EXP_SCALE,
                        )
                        if i < (6 if c == 1 else 2):
                            # these tiles also get an fp8 hi/lo split
                            # (s: 0=lo 1=hi) for the DoubleRow attn @ X;
                            # only the causally-valid columns [off:]
                            nc.gpsimd.tensor_copy(
                                at8[:, i, 1, off:], at[:, off:]
                            )
                            nc.gpsimd.tensor_tensor(
                                at8[:, i, 0, off:], at[:, off:],
                                at8[:, i, 1, off:], ALU.subtract,
                            )
                        tiles.append(at)
                    at_tiles[c] = (tiles, offs, at8)

                def run_attn_x(c):  # phase C (bf16) -> ot hi/lo fp8
                    # Rows 0-127 (c == 0, jj == 0) have tiny softmax denoms:
                    # after the final 1/rowsum scaling, the fp8 hi/lo
                    # subnormal floor on Z' would blow up relatively, so that
                    # 128-column slice also gets a bf16 copy and its output
                    # groups run as plain bf16 matmuls.
                    tiles, offs, at8 = at_tiles[c]
                    ot = atm.tile([P, ND, 2, CH], FP8, tag="ot_sb", bufs=2)
                    otb = None
                    if c == 0:
                        otb = atm.tile([P, ND, P], BF16, tag="ot_b16", bufs=1)
                    for dd in range(ND):
                        ps = psp.tile([P, CH], F32, tag="ot")

                        def dr_pair_partial(f, off2, skip_b0=False):
                            # full tile f paired with diagonal tile f+1:
                            # hi*hi of both over the overlap [off2:), cross
                            # terms per tile over each tile's valid range,
                            # and a plain-fp8 matmul for f's hi*hi head
                            nc.tensor.matmul(
                                ps[:, off2:],
                                xnat8_sb[:, f : f + 2, 0, ts(dd, P)],
                                at8[:, f : f + 2, 1, off2:CH],
                                start=False, stop=False, perf_mode=DR,
                            )
                            if not skip_b0:
                                nc.tensor.matmul(
                                    ps,
                                    xnat8_sb[:, f, 0:2, ts(dd, P)],
                                    at8[:, f, 0:2, :],
                                    start=False, stop=False, perf_mode=DR,
                                )
                            nc.tensor.matmul(
                                ps[:, off2:],
                                xnat8_sb[:, f + 1, 0:2, ts(dd, P)],
                                at8[:, f + 1, 0:2, off2:CH],
                                start=False, stop=False, perf_mode=DR,
                            )
                            nc.tensor.matmul(
                                ps[:, 0:off2],
                                xnat8_sb[:, f, 0, ts(dd, P)],
                                at8[:, f, 1, 0:off2],
                                start=False, stop=False,
                            )

                        if c == 1:
                            # tiles 0-3 (full width): fp8 hi/lo DoubleRow
                            for k, pr in enumerate(((0, 1), (2, 3))):
                                nc.tensor.matmul(
                                    ps,
                                    xnat8_sb[:, pr[0] : pr[0] + 2, 0, ts(dd, P)],
                                    at8[:, pr[0] : pr[0] + 2, 1, :],
                                    start=(k == 0), stop=False, perf_mode=DR,
                                )
                            for i in range(4):
                                nc.tensor.matmul(
                                    ps,
                                    xnat8_sb[:, i, 0:2, ts(dd, P)],
                                    at8[:, i, 0:2, :],
                                    start=False, stop=False, perf_mode=DR,
                                )
                            dr_pair_partial(4, offs[5])
                        else:
                            nc.tensor.matmul(
                                ps,
                                xnat8_sb[:, 0, 0:2, ts(dd, P)],
                                at8[:, 0, 0:2, :],
                                start=True, stop=False, perf_mode=DR,
                            )
                            dr_pair_partial(0, offs[1], skip_b0=True)
                        rng = range(6, ND) if c == 1 else range(2, 4)
                        for i in rng:
                            off = offs[i]
                            nc.tensor.matmul(
                                ps[:, off:],
                                xnat_sb[:, i, ts(dd, P)],
                                tiles[i][:, off:],
                                start=(i == 0 and c == 0),
                                stop=(i == len(tiles) - 1),
                            )
                        nc.scalar.activation(
                            ot[:, dd, 0, :], ps, AF.Identity,
                            bias=0.0, scale=Z_SCALE,
                        )
                        nc.vector.scalar_tensor_tensor(
                            ot[:, dd, 1, :], ps, Z_SCALE, ot[:, dd, 0, :],
                            ALU.mult, ALU.subtract,
                        )
                        if c == 0:
                            nc.scalar.activation(
                                otb[:, dd, :], ps[:, :P], AF.Identity,
                                bias=0.0, scale=Z_SCALE,
                            )
                    return ot, otb

                def run_rowsums(c):
                    # psum[tq, 0] = ONES_VAL * sum_tk at'[tk, tq], per
                    # tq-tile, summing exactly the values attn @ X consumes
                    # (the fp8 hi/lo pair for the chunk-1 DoubleRow tiles)
                    tiles, _, at8 = at_tiles[c]
                    ps_r = psp.tile([P, 4], F32, tag="rps", bufs=1)
                    for jj in range(4):
                        j = 4 * c + jj
                        for i in range(j + 1):
                            if i < (6 if c == 1 else 2):
                                nc.tensor.matmul(
                                    ps_r[:, jj : jj + 1],
                                    at8[:, i, 0:2, ts(jj, P)],
                                    ones8[:, 0:2, 0:1],
                                    start=(jj == 0 and i == 0),
                                    stop=(jj == 3 and i == j),
                                    perf_mode=DR,
                                )
                            else:
                                nc.tensor.matmul(
                                    ps_r[:, jj : jj + 1],
                                    tiles[i][:, ts(jj, P)],
                                    ones_c[:, 0:1],
                                    start=(jj == 0 and i == 0),
                                    stop=(jj == 3 and i == j),
                                )
                    nc.vector.reciprocal(r_all[:, 4 * c : 4 * c + 4], ps_r)

                def run_out_proj(c, ot, otb):  # phase D
                    for jj in range(4):
                        j = 4 * c + jj
                        for g in range(NCH):
                            # split the very last piece so the final stt+DMA
                            # epilogue chain pipelines under the matmuls
                            last = c == NCH - 1 and jj == 3
                            nh, w = (2, CH // 2) if last else (1, CH)
                            for h in range(nh):
                                lo = CH * g + w * h
                                ps = psp.tile([P, w], F32, tag="mm512", bufs=3)

                                if c == 0 and jj == 0:
                                    for dd in range(ND):
                                        nc.tensor.matmul(
                                            ps,
                                            otb[:, dd, :],
                                            ct_b16[:, dd, lo : lo + w],
                                            start=(dd == 0),
                                            stop=(dd == ND - 1),
                                        )
                                else:
                                    def stat(kind, d, _jj=jj):
                                        if kind == "hh":
                                            return ot[:, d : d + 2, 0, ts(_jj, P)]
                                        return ot[:, d, 0:2, ts(_jj, P)]

                                    def mov(kind, d, _lo=lo, _hi=lo + w):
                                        if kind == "hh":
                                            return ct_sb[:, d : d + 2, 1, _lo:_hi]
                                        return ct_sb[:, d, 0:2, _lo:_hi]

                                    dr_group(ps, stat, mov)
                                res = atm.tile([P, w], F32, tag="res", bufs=3)
                                nc.vector.scalar_tensor_tensor(
                                    res,
                                    ps,
                                    r_all[:, j : j + 1],
                                    bob[:, lo : lo + w],
                                    ALU.mult,
                                    ALU.add,
                                )
                                # chunk-1 results go out on the Pool lane
                                # (idle by then) except the very last piece,
                                # which rides SP so the two final stores
                                # drain in parallel
                                dma = nc.sync if c == 0 or (last and h == 1) \
                                    else nc.gpsimd
                                dma.dma_start(
                                    out_d[ts(j, P), lo : lo + w], res
                                )

                # PE-stream order chosen so cross-engine epilogue latencies
                # (y split after A, exp after B, ot split after C) hide under
                # the next PE block instead of stalling it.
                run_yproj(0)
                run_scores(0)
                run_yproj(1)
                ot0, otb0 = run_attn_x(0)
                run_rowsums(0)
                run_scores(1)
                run_out_proj(0, ot0, otb0)
                ot1, otb1 = run_attn_x(1)
                run_rowsums(1)
                run_out_proj(1, ot1, otb1)

    nc.finalize()
    return nc


def _legalize_waits(nc):
    """Hoist excess sync waits into preceding EventSemaphore instructions.

    The TRN2 ISA allows 1 inline sync-wait per engine instruction (2 for
    EventSemaphore); Tile can emit more (e.g. at pool-reuse boundaries), which
    walrus rejects with "Too many sync wait commands". An EventSemaphore on
    the same engine immediately before the instruction is semantically
    identical: the engine's sequencer blocks on it in program order.
    """
    import concourse.mybir as mybir
    import bass_rust as _bass_rust

    counter = 0
    for f in nc.m.functions:
        for bb in f.blocks:
            out = []
            changed = False
            for inst in bb.instructions:
                si = inst.sync_info
                ws = list(si.on_wait) if si and si.on_wait else []
                cap = 2 if inst.opcode == "EventSemaphore" else 1
                if len(ws) > cap:
                    extra, keep = ws[:-cap], ws[-cap:]
                    for i in range(0, len(extra), 2):
                        es = mybir.InstEventSemaphore(
                            name=f"I-eswait-{counter}", ins=[], outs=[]
                        )
                        counter += 1
                        es.engine = inst.engine
                        es.sync_info = _bass_rust.SyncInfo(
                            on_wait=extra[i : i + 2], on_update=[]
                        )
                        out.append(es)
                    si.on_wait = keep
                    inst.sync_info = si
                    changed = True
                out.append(inst)
            if changed:
                bb.instructions = out
    return counter


def _get_program():
    if "nc" not in _CACHE:
        _CACHE["nc"] = _build_program()
    return _CACHE["nc"]


def _split8(a):
    hi = np.clip(a, -224.0, 224.0).astype(F8)
    lo = (a - hi.astype(np.float32)).astype(F8)
    return hi, lo


def _prep_shared(w_q, b_q, w_k, b_k, w_v, b_v, w_o, b_o):
    f = np.float32
    w_q, b_q = np.asarray(w_q, f), np.asarray(b_q, f)
    w_k, b_k = np.asarray(w_k, f), np.asarray(b_k, f)
    w_v, b_v = np.asarray(w_v, f), np.asarray(b_v, f)
    w_o, b_o = np.asarray(w_o, f), np.asarray(b_o, f)

    a32 = (w_q.T @ w_k) * f(A_SCALE)                 # 16 Wq^T Wk  [d, d']
    u32 = (w_k.T @ b_q) * f(A_SCALE)                 # 16 Wk^T bq  [d']
    ct32 = (w_o @ w_v).T * f(C_SCALE)                # 32 C^T      [d, e]
    bop = w_o @ b_v + b_o                            # b'          [e]

    # a32t[ee, p, dd, s, el] = split_s(a32)[128 dd + p, 128 ee + el], s 0=lo
    ahi, alo = _split8(a32.reshape(ND, P, ND, P).transpose(2, 1, 0, 3))
    ctr = ct32.reshape(ND, P, D)                     # [dd, p, e]
    cthi, ctlo = _split8(ctr)
    shared = {
        "a32t": np.ascontiguousarray(np.stack([alo, ahi], axis=3)),
        "ct32": np.ascontiguousarray(np.stack([ctlo, cthi], axis=2)),
        "ctb": np.ascontiguousarray(ctr.transpose(1, 0, 2)).astype(BF),
        "u32T": np.ascontiguousarray(u32.reshape(ND, P).T),
        "bob": np.ascontiguousarray(
            np.broadcast_to(bop[None, :], (P, D))
        ).astype(BF),
    }
    ii = np.arange(P)
    shared["maskT"] = np.where(
        ii[:, None] <= ii[None, :], f(0.0), f(MASK_VAL)
    ).astype(f)
    return shared


def kernel(x, w_q, b_q, w_k, b_k, w_v, b_v, w_o, b_o):
    from concourse.bass_utils import run_bass_kernel_spmd

    nc = _get_program()
    if not _CACHE.get("legalized"):
        _legalize_waits(nc)
        _CACHE["legalized"] = True
    shared = _prep_shared(w_q, b_q, w_k, b_k, w_v, b_v, w_o, b_o)
    x = np.asarray(x, np.float32)
    in_maps = []
    for b in range(B):
        xb = x[b]
        # xT8[c, s, p, dd, t] = split_s(xb.T)[128 dd + p, 512 c + t]
        xt = xb.T.reshape(ND, P, NCH, CH).transpose(2, 1, 0, 3)  # [c, p, dd, t]
        xhi, xlo = _split8(xt)
        m = dict(shared)
        m["xT8"] = np.ascontiguousarray(np.stack([xhi, xlo], axis=3))
        m["xnat"] = np.ascontiguousarray(
            xb.reshape(NT, P, D).transpose(1, 0, 2)
        ).astype(BF)
        xn = xb.reshape(NT, P, D).transpose(1, 0, 2)[:, :6]  # [p, j, e]
        xnhi, xnlo = _split8(xn)
        m["xnat8"] = np.ascontiguousarray(np.stack([xnhi, xnlo], axis=2))
        in_maps.append(m)

    trace = bool(os.environ.get("KERNEL_TRACE"))
    try:
        res = run_bass_kernel_spmd(nc, in_maps, list(range(B)), trace=trace)
    except ModuleNotFoundError:
        # axon NTFF profile hook not present in this container; rerun with
        # tracing disabled rather than failing the kernel call.
        os.environ["BASS_NEVER_TRACE"] = "1"
        res = run_bass_kernel_spmd(nc, in_maps, list(range(B)), trace=False)
    _CACHE["last_results"] = res
    out = np.stack([res.results[b]["out"] for b in range(B)], axis=0)
    return out


# revision 50
# speedup vs baseline: 1.0162x; 1.0162x over previous
"""Causal single-head attention block on 8 TRN2 NeuronCores.

Problem: B=8, T=1024, D=1024 fp32.
    q = x @ w_q.T + b_q ; k, v likewise
    scores = (q @ k.T) / sqrt(D), causal mask, softmax
    out = (softmax @ v) @ w_o.T + b_o

Sharding: pure data-parallel - core c computes batch element c. Weights are
replicated. No collectives.

Algebraic restructuring (removes 2 of the 5 D x D GEMMs):
  scores_ij = q_i . k_j = x_i^T (Wq^T Wk) x_j + (Wk^T bq) . x_j + f(i)
  where f(i) is constant per query row and cancels in softmax. So with
  A = Wq^T Wk and u = Wk^T bq (precomputed host-side from the weights):
      Y = X A + 1 u^T        (one GEMM; replaces the q AND k projections)
      S = Y X^T / sqrt(D)    (the score GEMM, X itself is the stationary side)
  Similarly, since softmax rows sum to 1, the v bias feeds straight through:
      out = attn @ (X Wv^T + 1 bv^T) @ Wo^T + 1 bo^T
          = (attn @ X) @ C^T + 1 b'^T,  C = Wo Wv, b' = Wo bv + bo.
  Total per-core MACs drop from 5.36e9 to 3.28e9.

fp8 hi/lo DoubleRow matmuls (~1.33x on the remaining big GEMMs):
  Every big-GEMM operand is split as w = hi + lo with hi = fp8(w),
  lo = fp8(w - hi); dropping only the lo*lo term, each 128-deep contraction
  chunk needs 3 fp8 sub-matmuls, packed 2-per-instruction with
  MatmulPerfMode.DoubleRow (0.5 PE cycles/row):
      DR_A(d,d+1) = hi_d*hi_d + hi_{d+1}*hi_{d+1}   (cross-chunk pair)
      DR_B(d)     = hi_d*lo_d + lo_d*hi_d           (within-chunk pair)
  so 8 chunks take 12 DR instructions = 6N cycles vs 8N for bf16, at
  bf16-level accuracy. Scaling (all folded away exactly by the ONES_VAL
  rowsum constant and the exp scale): A is pre-scaled by 16 (|16 Y'| < 123
  across all 8 batch inputs, vs the e4m3 max of 240) and C by 32 (fp8
  subnormal headroom); the exp() epilogue absorbs 1/16/sqrt(D) into its
  scale and an extra -ln(32) bias plus a 1/4 scale on the Z' = attn @ X
  hi/lo split keep exp outputs and Z' inside e4m3 range (|Z'/4| < 97).

  attn weights are split hi/lo only for the full-width (non-diagonal)
  chunk-1 tiles, where DoubleRow pairs exist; the softmax denominators for
  those tiles sum the same hi+lo pair via a DoubleRow ones-matmul, so the
  normalization stays exactly consistent with the numerator. Plain-fp8
  (non-split) attn weights would NOT work: softmax normalization does not
  attenuate their quantization error and the error budget blows.

  Rows 0-127 have tiny softmax denominators; after the final 1/rowsum
  scaling the fp8 subnormal floor on Z' would blow up relatively, so that
  row block runs through a parallel bf16 path (otb/ctb) instead.

Device-side layout strategy (transpose-free), as in the baseline:
  - scores are computed TRANSPOSED: scoresT[tk, tq], k-side stationary.
  - softmax denominators via tiny N=1 matmuls against a constant column.
  - causal structure: for tq-chunk c (512 wide), only tk-tiles i <= 4c+3 are
    computed; partially-valid tiles use shortened matmuls and the diagonal
    128x128 block gets an additive -1e30 upper-tri mask.
"""

import os
import numpy as np
import ml_dtypes

BF = ml_dtypes.bfloat16
F8 = ml_dtypes.float8_e4m3

B, T, D = 8, 1024, 1024
P = 128
ND = D // P          # 8 d-tiles / e-tiles
NT = T // P          # 8 t-tiles
CH = 512             # matmul moving free-dim (one PSUM bank of fp32)
NCH = T // CH        # 2 tq-chunks
SM_SCALE = float(D) ** -0.5
A_SCALE = 16.0       # host pre-scale on A = Wq^T Wk (fp8 range: |16 Y'| < 123)
C_SCALE = 32.0       # host pre-scale on C = Wo Wv (fp8 subnormal headroom)
Z_SCALE = 0.25       # extra scale on the Z' fp8 hi/lo split (|Z'/4| < 97)
ONES_VAL = 8.0       # rowsum column constant; folds all scales away exactly
LN32 = 3.4657359027997265
EXP_SCALE = SM_SCALE / A_SCALE   # 1/512, applied inside the exp activation
MASK_VAL = -1.0e30

_CACHE = {}


def _build_program():
    import concourse.bass as bass
    import concourse.mybir as mybir
    import concourse.tile as tile
    from concourse.bass import ts

    F32 = mybir.dt.float32
    BF16 = mybir.dt.bfloat16
    FP8 = mybir.dt.float8e4
    AF = mybir.ActivationFunctionType
    ALU = mybir.AluOpType
    DR = mybir.MatmulPerfMode.DoubleRow

    nc = bass.Bass()

    # x arrives pre-split/pre-tiled from the host:
    #   xT8[c, p, dd, s, t] = split_s(x.T)[128 dd + p, 512 c + t], s: 0=hi 1=lo
    #   xnat[p, j, e] = x[128 j + p, e] (bf16, stationary side of attn @ X)
    xT8_d = nc.declare_dram_parameter("xT8", [NCH, P, ND, 2, CH], FP8, isOutput=False)
    xnat_d = nc.declare_dram_parameter("xnat", [P, NT, D], BF16, isOutput=False)
    # fp8 hi/lo copy of x rows 0-767 (stationary side of the DoubleRow
    # attn @ X tiles); s: 0=hi 1=lo
    xnat8_d = nc.declare_dram_parameter("xnat8", [P, 6, 2, D], FP8, isOutput=False)
    # A32t[ee, p, dd, s, el] = split_s(32 Wq^T Wk)[128 dd + p, 128 ee + el],
    # s: 0=lo 1=hi  (one contiguous 2 KiB run per partition per ee)
    a_d = nc.declare_dram_parameter("a32t", [ND, P, ND, 2, P], FP8, isOutput=False)
    # ct32[dd, p, s, e] = split_s(32 (Wo Wv)^T)[128 dd + p, e], s: 0=lo 1=hi
    ct_d = nc.declare_dram_parameter("ct32", [ND, P, 2, D], FP8, isOutput=False)
    # bf16 copy of 32 C^T for the early-row (c=0, jj=0) output groups
    ctb_d = nc.declare_dram_parameter("ctb", [P, ND, D], BF16, isOutput=False)
    u32_d = nc.declare_dram_parameter("u32T", [P, ND], F32, isOutput=False)
    bob_d = nc.declare_dram_parameter("bob", [P, D], BF16, isOutput=False)
    mask_d = nc.declare_dram_parameter("maskT", [P, P], F32, isOutput=False)
    out_d = nc.declare_dram_parameter("out", [T, D], F32, isOutput=True)

    # Default slot order touches chunks 6/7 last: their operands are the
    # ones still in flight (last DMA piece, or the epilogue of the previous
    # phase's final tile), so the group starts without waiting on them.
    DR_ORDER = [("hh", 0), ("hh", 2), ("hh", 4), ("x", 0), ("x", 1),
                ("x", 2), ("x", 3), ("x", 4), ("x", 5), ("hh", 6),
                ("x", 6), ("x", 7)]
    # For the very first group the xT chunk-0 pieces land dd-pair by dd-pair
    # across both DMA lanes: consume in arrival order.
    DR_ORDER_FIRST = [("hh", 0), ("x", 0), ("x", 1), ("hh", 6), ("x", 6),
                      ("x", 7), ("hh", 2), ("x", 2), ("x", 3), ("hh", 4),
                      ("x", 4), ("x", 5)]

    def dr_group(ps, stat, mov, order=None):
        """Emit the 12-instruction DoubleRow group for one 1024-deep
        contraction; stat/mov are callables returning the (2, free)-shaped
        slot APs:
            stat('hh', d) / mov('hh', d): hi slots of chunks d and d+1
            stat('x', d)  / mov('x', d) : the two cross slots of chunk d
        """
        order = DR_ORDER if order is None else order
        for k, (kind, d) in enumerate(order):
            nc.tensor.matmul(ps, stat(kind, d), mov(kind, d),
                             start=(k == 0), stop=(k == len(order) - 1),
                             perf_mode=DR)

    with tile.TileContext(nc) as tc:
        with (
            tc.tile_pool(name="pers", bufs=1) as pers,
            tc.tile_pool(name="psum", bufs=2, space="PSUM") as psp,
        ):
            # ---- persistent SBUF tensors --------------------------------
            xT_sb = pers.tile([P, ND, 2, T], FP8)     # s: 0=hi 1=lo
            xnat_sb = pers.tile([P, NT, D], BF16)
            xnat8_sb = pers.tile([P, 6, 2, D], FP8)   # s: 0=hi 1=lo
            y_sb = pers.tile([P, ND, 2, T], FP8)      # s: 0=lo 1=hi
            a_sb = pers.tile([P, ND, ND, 2, P], FP8)  # [p, ee, dd, s, el] 0=lo 1=hi
            ct_sb = pers.tile([P, ND, 2, D], FP8)     # s: 0=lo 1=hi
            ct_b16 = pers.tile([P, ND, D], BF16)
            u32 = pers.tile([P, ND], F32)
            bob = pers.tile([P, D], BF16)
            maskT = pers.tile([P, P], F32)
            ones_c = pers.tile([P, 2], BF16)
            ones8 = pers.tile([P, 2, 1], FP8)
            negln32 = pers.tile([P, 1], F32)
            r_all = pers.tile([P, NT], F32)

            with tc.tile_pool(name="attn_tmp", bufs=3) as atm:
                # ---- DMAs: two serial lanes (SP sequencer + Pool/SWDGE).
                # Critical path: A[ee=0] + xT hi chunk 0 feed the first
                # DoubleRow group.
                # xT chunk 0 lands as dd-pair pieces (hi+lo together) split
                # across both DMA lanes, in the order DR_ORDER_FIRST consumes
                # them; DMA latency is ~1.9 us flat so small leading pieces
                # start the first DoubleRow group ~1 us earlier.
                nc.sync.dma_start(a_sb[:, 0], a_d[0])
                nc.gpsimd.dma_start(
                    xT_sb[:, 0:2, :, ts(0, CH)], xT8_d[0][:, 0:2]
                )
                nc.sync.dma_start(
                    xT_sb[:, 6:ND, :, ts(0, CH)], xT8_d[0][:, 6:ND]
                )
                nc.gpsimd.dma_start(
                    xT_sb[:, 2:4, :, ts(0, CH)], xT8_d[0][:, 2:4]
                )
                nc.gpsimd.dma_start(
                    xT_sb[:, 4:6, :, ts(0, CH)], xT8_d[0][:, 4:6]
                )
                nc.sync.dma_start(u32, u32_d[:, :])
                for ee in range(1, 4):
                    nc.sync.dma_start(a_sb[:, ee], a_d[ee])
                nc.gpsimd.dma_start(xT_sb[:, :, 0, ts(1, CH)], xT8_d[1][:, :, 0])
                nc.gpsimd.dma_start(xT_sb[:, :, 1, ts(1, CH)], xT8_d[1][:, :, 1])
                for ee in range(4, ND):
                    nc.gpsimd.dma_start(a_sb[:, ee], a_d[ee])
                nc.sync.dma_start(maskT, mask_d[:, :])
                for dd in range(ND):
                    nc.sync.dma_start(ct_sb[:, dd], ct_d[dd])
                nc.sync.dma_start(bob, bob_d[:, :])
                # xnat/ctb ride the SP lane: Pool must drain before the
                # chunk-0 at8 splits (Pool compute) gate attn_x(0)
                nc.gpsimd.dma_start(xnat8_sb, xnat8_d[:, :])
                nc.sync.dma_start(xnat_sb, xnat_d[:, :])
                nc.sync.dma_start(ct_b16, ctb_d[:, :])


                # ---- PE warm-up: dummy matmuls overlap the initial DMA
                # fill and spin the p-state clock up; result never read. The
                # dummy activation pre-loads the ScalarE exp table (the set
                # containing exp also contains identity, so no reloads).
                warm_in = atm.tile([P, P], BF16, tag="warm", bufs=1)
                nc.vector.memset(warm_in, 0.0)
                nc.vector.memset(ones_c, float(ONES_VAL))
                nc.vector.memset(ones8, float(ONES_VAL))
                nc.vector.memset(negln32, -LN32)
                act_warm = atm.tile([P, 1], F32, tag="warma", bufs=1)
                nc.scalar.activation(
                    act_warm, warm_in[:, :1], AF.Exp, bias=0.0, scale=1.0
                )
                warm_ps = psp.tile([P, CH], F32, tag="mm512", bufs=3)
                for _ in range(15):
                    nc.tensor.matmul(
                        warm_ps[:, :P], warm_in, warm_in, start=True, stop=True
                    )

                # ---- phase A: Y = 32(X A + 1 u^T), split hi/lo ----------
                def a_stat(ee):
                    def f(kind, d):
                        if kind == "hh":
                            return a_sb[:, ee, d : d + 2, 1, :]
                        return a_sb[:, ee, d, 0:2, :]
                    return f

                def xmov(c):
                    lo, hi = CH * c, CH * (c + 1)
                    def f(kind, d):
                        if kind == "hh":
                            return xT_sb[:, d : d + 2, 0, lo:hi]
                        return xT_sb[:, d, 0:2, lo:hi]
                    return f

                def run_yproj(c):
                    for ee in range(ND):
                        ps = psp.tile([P, CH], F32, tag="mm512", bufs=3)
                        dr_group(ps, a_stat(ee), xmov(c),
                                 DR_ORDER_FIRST if c == 0 and ee == 0 else None)
                        # yhi then ylo = (ps + u) - yhi; PSUM -> SBUF fp8
                        nc.scalar.activation(
                            y_sb[:, ee, 1, ts(c, CH)],
                            ps,
                            AF.Identity,
                            bias=u32[:, ee : ee + 1],
                            scale=1.0,
                        )
                        nc.vector.scalar_tensor_tensor(
                            y_sb[:, ee, 0, ts(c, CH)],
                            ps,
                            u32[:, ee : ee + 1],
                            y_sb[:, ee, 1, ts(c, CH)],
                            ALU.add,
                            ALU.subtract,
                        )
                        if c == 0 and ee == 0:
                            # bridge until the xT lo-half / chunk-1 DMAs land
                            for _ in range(6):
                                nc.tensor.matmul(
                                    warm_ps[:, :P],
                                    warm_in,
                                    warm_in,
                                    start=True,
                                    stop=True,
                                )

                # ---- phases B/C/D per tq-chunk --------------------------
                at_tiles = {}

                def run_scores(c):  # phase B
                    n_tk = 4 * (c + 1)
                    at8 = atm.tile([P, 6, 2, CH], FP8, tag="at8", bufs=2)
                    tiles, offs = [], []
                    for i in range(n_tk):
                        off = max(0, P * i - CH * c)
                        offs.append(off)
                        lo, hi = CH * c + off, CH * (c + 1)
                        ps = psp.tile([P, CH], F32, tag="sc", bufs=2)

                        def stat(kind, d, _i=i):
                            if kind == "hh":
                                return xT_sb[:, d : d + 2, 0, ts(_i, P)]
                            return xT_sb[:, d, 0:2, ts(_i, P)]

                        def mov(kind, d, _lo=lo, _hi=hi):
                            if kind == "hh":
                                return y_sb[:, d : d + 2, 1, _lo:_hi]
                            return y_sb[:, d, 0:2, _lo:_hi]

                        dr_group(ps[:, off:], stat, mov)
                        if i >= 4 * c:
                            # diagonal 128x128 block: additive upper-tri mask
                            nc.vector.tensor_add(
                                ps[:, off : off + P], ps[:, off : off + P], maskT
                            )
                        at = atm.tile([P, CH], BF16, tag="at", bufs=9)
                        nc.scalar.activation(
                            at[:, off:], ps[:, off:], AF.Exp,
                            bias=negln32[:, 0:1], scale=EXP_SCALE,
                        )
                        if i < (6 if c == 1 else 2):
                            # these tiles also get an fp8 hi/lo split
                            # (s: 0=lo 1=hi) for the DoubleRow attn @ X;
                            # only the causally-valid columns [off:]
                            nc.gpsimd.tensor_copy(
                                at8[:, i, 1, off:], at[:, off:]
                            )
                            nc.gpsimd.tensor_tensor(
                                at8[:, i, 0, off:], at[:, off:],
                                at8[:, i, 1, off:], ALU.subtract,
                            )
                        tiles.append(at)
                    at_tiles[c] = (tiles, offs, at8)

                def run_attn_x(c):  # phase C (bf16) -> ot hi/lo fp8
                    # Rows 0-127 (c == 0, jj == 0) have tiny softmax denoms:
                    # after the final 1/rowsum scaling, the fp8 hi/lo
                    # subnormal floor on Z' would blow up relatively, so that
                    # 128-column slice also gets a bf16 copy and its output
                    # groups run as plain bf16 matmuls.
                    tiles, offs, at8 = at_tiles[c]
                    ot = atm.tile([P, ND, 2, CH], FP8, tag="ot_sb", bufs=2)
                    otb = None
                    if c == 0:
                        otb = atm.tile([P, ND, P], BF16, tag="ot_b16", bufs=1)
                    for dd in range(ND):
                        ps = psp.tile([P, CH], F32, tag="ot")

                        def dr_pair_partial(f, off2, skip_b0=False):
                            # full tile f paired with diagonal tile f+1:
                            # hi*hi of both over the overlap [off2:), cross
                            # terms per tile over each tile's valid range,
                            # and a plain-fp8 matmul for f's hi*hi head
                            nc.tensor.matmul(
                                ps[:, off2:],
                                xnat8_sb[:, f : f + 2, 0, ts(dd, P)],
                                at8[:, f : f + 2, 1, off2:CH],
                                start=False, stop=False, perf_mode=DR,
                            )
                            if not skip_b0:
                                nc.tensor.matmul(
                                    ps,
                                    xnat8_sb[:, f, 0:2, ts(dd, P)],
                                    at8[:, f, 0:2, :],
                                    start=False, stop=False, perf_mode=DR,
                                )
                            nc.tensor.matmul(
                                ps[:, off2:],
                                xnat8_sb[:, f + 1, 0:2, ts(dd, P)],
                                at8[:, f + 1, 0:2, off2:CH],
                                start=False, stop=False, perf_mode=DR,
                            )
                            nc.tensor.matmul(
                                ps[:, 0:off2],
                                xnat8_sb[:, f, 0, ts(dd, P)],
                                at8[:, f, 1, 0:off2],
                                start=False, stop=False,
                            )

                        if c == 1:
                            # tiles 0-3 (full width): fp8 hi/lo DoubleRow
                            for k, pr in enumerate(((0, 1), (2, 3))):
                                nc.tensor.matmul(
                                    ps,
                                    xnat8_sb[:, pr[0] : pr[0] + 2, 0, ts(dd, P)],
                                    at8[:, pr[0] : pr[0] + 2, 1, :],
                                    start=(k == 0), stop=False, perf_mode=DR,
                                )
                            for i in range(4):
                                nc.tensor.matmul(
                                    ps,
                                    xnat8_sb[:, i, 0:2, ts(dd, P)],
                                    at8[:, i, 0:2, :],
                                    start=False, stop=False, perf_mode=DR,
                                )
                            dr_pair_partial(4, offs[5])
                        else:
                            nc.tensor.matmul(
                                ps,
                                xnat8_sb[:, 0, 0:2, ts(dd, P)],
                                at8[:, 0, 0:2, :],
                                start=True, stop=False, perf_mode=DR,
                            )
                            dr_pair_partial(0, offs[1], skip_b0=True)
                        rng = range(6, ND) if c == 1 else range(2, 4)
                        for i in rng:
                            off = offs[i]
                            nc.tensor.matmul(
                                ps[:, off:],
                                xnat_sb[:, i, ts(dd, P)],
                                tiles[i][:, off:],
                                start=(i == 0 and c == 0),
                                stop=(i == len(tiles) - 1),
                            )
                        nc.scalar.activation(
                            ot[:, dd, 0, :], ps, AF.Identity,
                            bias=0.0, scale=Z_SCALE,
                        )
                        nc.vector.scalar_tensor_tensor(
                            ot[:, dd, 1, :], ps, Z_SCALE, ot[:, dd, 0, :],
                            ALU.mult, ALU.subtract,
                        )
                        if c == 0:
                            nc.scalar.activation(
                                otb[:, dd, :], ps[:, :P], AF.Identity,
                                bias=0.0, scale=Z_SCALE,
                            )
                    return ot, otb

                def run_rowsums(c):
                    # psum[tq, 0] = ONES_VAL * sum_tk at'[tk, tq], per
                    # tq-tile, summing exactly the values attn @ X consumes
                    # (the fp8 hi/lo pair for the chunk-1 DoubleRow tiles)
                    tiles, _, at8 = at_tiles[c]
                    ps_r = psp.tile([P, 4], F32, tag="rps", bufs=1)
                    for jj in range(4):
                        j = 4 * c + jj
                        for i in range(j + 1):
                            if i < (6 if c == 1 else 2):
                                nc.tensor.matmul(
                                    ps_r[:, jj : jj + 1],
                                    at8[:, i, 0:2, ts(jj, P)],
                                    ones8[:, 0:2, 0:1],
                                    start=(jj == 0 and i == 0),
                                    stop=(jj == 3 and i == j),
                                    perf_mode=DR,
                                )
                            else:
                                nc.tensor.matmul(
                                    ps_r[:, jj : jj + 1],
                                    tiles[i][:, ts(jj, P)],
                                    ones_c[:, 0:1],
                                    start=(jj == 0 and i == 0),
                                    stop=(jj == 3 and i == j),
                                )
                    nc.vector.reciprocal(r_all[:, 4 * c : 4 * c + 4], ps_r)

                def run_out_proj(c, ot, otb):  # phase D
                    for jj in range(4):
                        j = 4 * c + jj
                        for g in range(NCH):
                            # split the very last piece so the final stt+DMA
                            # epilogue chain pipelines under the matmuls
                            last = c == NCH - 1 and jj == 3
                            nh, w = (2, CH // 2) if last else (1, CH)
                            for h in range(nh):
                                lo = CH * g + w * h
                                ps = psp.tile([P, w], F32, tag="mm512", bufs=3)

                                if c == 0 and jj == 0:
                                    for dd in range(ND):
                                        nc.tensor.matmul(
                                            ps,
                                            otb[:, dd, :],
                                            ct_b16[:, dd, lo : lo + w],
                                            start=(dd == 0),
                                            stop=(dd == ND - 1),
                                        )
                                else:
                                    def stat(kind, d, _jj=jj):
                                        if kind == "hh":
                                            return ot[:, d : d + 2, 0, ts(_jj, P)]
                                        return ot[:, d, 0:2, ts(_jj, P)]

                                    def mov(kind, d, _lo=lo, _hi=lo + w):
                                        if kind == "hh":
                                            return ct_sb[:, d : d + 2, 1, _lo:_hi]
                                        return ct_sb[:, d, 0:2, _lo:_hi]

                                    dr_group(ps, stat, mov)
                                res = atm.tile([P, w], F32, tag="res", bufs=3)
                                nc.vector.scalar_tensor_tensor(
                                    res,
                                    ps,
                                    r_all[:, j : j + 1],
                                    bob[:, lo : lo + w],
                                    ALU.mult,
                                    ALU.add,
                                )
                                # chunk-1 results go out on the Pool lane
                                # (idle by then) except the very last piece,
                                # which rides SP so the two final stores
                                # drain in parallel
                                dma = nc.sync if c == 0 or (last and h == 1) \
                                    else nc.gpsimd
                                dma.dma_start(
                                    out_d[ts(j, P), lo : lo + w], res
                                )

                # PE-stream order chosen so cross-engine epilogue latencies
                # (y split after A, exp after B, ot split after C) hide under
                # the next PE block instead of stalling it.
                run_yproj(0)
                run_scores(0)
                run_yproj(1)
                ot0, otb0 = run_attn_x(0)
                run_rowsums(0)
                run_scores(1)
                run_out_proj(0, ot0, otb0)
                ot1, otb1 = run_attn_x(1)
                run_rowsums(1)
                run_out_proj(1, ot1, otb1)

    nc.finalize()
    return nc


def _legalize_waits(nc):
    """Hoist excess sync waits into preceding EventSemaphore instructions.

    The TRN2 ISA allows 1 inline sync-wait per engine instruction (2 for
    EventSemaphore); Tile can emit more (e.g. at pool-reuse boundaries), which
    walrus rejects with "Too many sync wait commands". An EventSemaphore on
    the same engine immediately before the instruction is semantically
    identical: the engine's sequencer blocks on it in program order.
    """
    import concourse.mybir as mybir
    import bass_rust as _bass_rust

    counter = 0
    for f in nc.m.functions:
        for bb in f.blocks:
            out = []
            changed = False
            for inst in bb.instructions:
                si = inst.sync_info
                ws = list(si.on_wait) if si and si.on_wait else []
                cap = 2 if inst.opcode == "EventSemaphore" else 1
                if len(ws) > cap:
                    extra, keep = ws[:-cap], ws[-cap:]
                    for i in range(0, len(extra), 2):
                        es = mybir.InstEventSemaphore(
                            name=f"I-eswait-{counter}", ins=[], outs=[]
                        )
                        counter += 1
                        es.engine = inst.engine
                        es.sync_info = _bass_rust.SyncInfo(
                            on_wait=extra[i : i + 2], on_update=[]
                        )
                        out.append(es)
                    si.on_wait = keep
                    inst.sync_info = si
                    changed = True
                out.append(inst)
            if changed:
                bb.instructions = out
    return counter


def _get_program():
    if "nc" not in _CACHE:
        _CACHE["nc"] = _build_program()
    return _CACHE["nc"]


def _split8(a):
    hi = np.clip(a, -224.0, 224.0).astype(F8)
    lo = (a - hi.astype(np.float32)).astype(F8)
    return hi, lo


def _prep_shared(w_q, b_q, w_k, b_k, w_v, b_v, w_o, b_o):
    f = np.float32
    w_q, b_q = np.asarray(w_q, f), np.asarray(b_q, f)
    w_k, b_k = np.asarray(w_k, f), np.asarray(b_k, f)
    w_v, b_v = np.asarray(w_v, f), np.asarray(b_v, f)
    w_o, b_o = np.asarray(w_o, f), np.asarray(b_o, f)

    a32 = (w_q.T @ w_k) * f(A_SCALE)                 # 16 Wq^T Wk  [d, d']
    u32 = (w_k.T @ b_q) * f(A_SCALE)                 # 16 Wk^T bq  [d']
    ct32 = (w_o @ w_v).T * f(C_SCALE)                # 32 C^T      [d, e]
    bop = w_o @ b_v + b_o                            # b'          [e]

    # a32t[ee, p, dd, s, el] = split_s(a32)[128 dd + p, 128 ee + el], s 0=lo
    ahi, alo = _split8(a32.reshape(ND, P, ND, P).transpose(2, 1, 0, 3))
    ctr = ct32.reshape(ND, P, D)                     # [dd, p, e]
    cthi, ctlo = _split8(ctr)
    shared = {
        "a32t": np.ascontiguousarray(np.stack([alo, ahi], axis=3)),
        "ct32": np.ascontiguousarray(np.stack([ctlo, cthi], axis=2)),
        "ctb": np.ascontiguousarray(ctr.transpose(1, 0, 2)).astype(BF),
        "u32T": np.ascontiguousarray(u32.reshape(ND, P).T),
        "bob": np.ascontiguousarray(
            np.broadcast_to(bop[None, :], (P, D))
        ).astype(BF),
    }
    ii = np.arange(P)
    shared["maskT"] = np.where(
        ii[:, None] <= ii[None, :], f(0.0), f(MASK_VAL)
    ).astype(f)
    return shared


def kernel(x, w_q, b_q, w_k, b_k, w_v, b_v, w_o, b_o):
    from concourse.bass_utils import run_bass_kernel_spmd

    nc = _get_program()
    if not _CACHE.get("legalized"):
        _legalize_waits(nc)
        _CACHE["legalized"] = True
    shared = _prep_shared(w_q, b_q, w_k, b_k, w_v, b_v, w_o, b_o)
    x = np.asarray(x, np.float32)
    in_maps = []
    for b in range(B):
        xb = x[b]
        # xT8[c, s, p, dd, t] = split_s(xb.T)[128 dd + p, 512 c + t]
        xt = xb.T.reshape(ND, P, NCH, CH).transpose(2, 1, 0, 3)  # [c, p, dd, t]
        xhi, xlo = _split8(xt)
        m = dict(shared)
        m["xT8"] = np.ascontiguousarray(np.stack([xhi, xlo], axis=3))
        m["xnat"] = np.ascontiguousarray(
            xb.reshape(NT, P, D).transpose(1, 0, 2)
        ).astype(BF)
        xn = xb.reshape(NT, P, D).transpose(1, 0, 2)[:, :6]  # [p, j, e]
        xnhi, xnlo = _split8(xn)
        m["xnat8"] = np.ascontiguousarray(np.stack([xnhi, xnlo], axis=2))
        in_maps.append(m)

    trace = bool(os.environ.get("KERNEL_TRACE"))
    try:
        res = run_bass_kernel_spmd(nc, in_maps, list(range(B)), trace=trace)
    except ModuleNotFoundError:
        # axon NTFF profile hook not present in this container; rerun with
        # tracing disabled rather than failing the kernel call.
        os.environ["BASS_NEVER_TRACE"] = "1"
        res = run_bass_kernel_spmd(nc, in_maps, list(range(B)), trace=False)
    _CACHE["last_results"] = res
    out = np.stack([res.results[b]["out"] for b in range(B)], axis=0)
    return out
